# revision 23
# baseline (speedup 1.0000x reference)
"""AttentionWithRotaryPositionalEmbedding — Trainium2 Bass kernel (v2).

Shapes (hardcoded, from the problem spec):
  x: (8, 2048, 512), mask: (8, 2048), W_qkv: (1536, 512),
  W_proj: (512, 512), b_proj: (512,), num_cls_token: scalar
Sharding: data-parallel over batch B=8 across the 8 NeuronCores; weights
replicated. No collectives.

Per-core dataflow (batch b):
  Stage A (k/v):  k = x_b @ W_k^T via PE (W_k host-permuted so each head is
    [un(32)|ev(32)] and pre-scaled by 2^23/ln2 * 0.125 for the softmax
    bit-trick); rope as 3 full-width DVE ops (mult, mult-on-swapped-view,
    add); PE transpose to kT[d, n]; v copied into an augmented [v|1] tile
    whose ones-column yields softmax denominators from the AV matmul.
  Per query chunk (512): same for q, then attention:
    scoresT[m, n] = k^T q    PE, two heads packed via row tiles (0,0)/(64,0)
    eT = exp(..)             key blocks 0..9: ScalarE exp (scale=1/A,
                             bias=mask); blocks 10..15: GPSIMD Schraudolph
                             (int32 add of A*mask+B, bitcast to fp32)
    num[d|den, n] += vaug^T eT   PE accumulate
    attnT = num * (1/den)    DVE reciprocal + gpsimd partition_broadcast
  outT = W_proj attnT + b_proj   PE + DVE bias (stored transposed; host
                                 transposes back)
"""

import numpy as np

import concourse.bass as bass
import concourse.tile as tile
from concourse import bacc, mybir
from concourse.alu_op_type import AluOpType
from concourse.masks import make_identity

P = 128
B = 8
N = 2048
C = 512
H = 8
D = 64
F = 3 * C          # 1536
NB = N // P        # 16 token blocks
CB = C // P        # 4 contraction chunks
PAIRS = H // 2     # 4 head pairs
NCH = N // 512     # 4 query chunks of 512
VW = H * (D + 1)   # 520

A_SCHR = float(2 ** 7) / float(np.log(2.0))    # bf16-space Schraudolph slope
C_SCHR = 366393.0 / 65536.0
B_SCHR = 127.0 * 2 ** 7 - C_SCHR
S_ACT = 12          # key blocks [0, S_ACT) use ScalarE exp; rest DVE Schraudolph

F32 = mybir.dt.float32
F32R = mybir.dt.float32r
BF16 = mybir.dt.bfloat16
I16 = mybir.dt.int16
EXP = mybir.ActivationFunctionType.Exp


def build_nc(repeats: int = 1, debug: bool = False, ablate: frozenset = frozenset()):
    """ablate: 'no_schr' -> all-ScalarE exp (error attribution);
    'no_b' / 'no_c' -> skip stages (timing only)."""
    s_act = NB if "no_schr" in ablate else S_ACT
    nc = bacc.Bacc("TRN2", target_bir_lowering=False, debug=False, num_devices=B)

    xT = nc.dram_tensor("xT", [NB, P, CB, P], F32R, kind="ExternalInput").ap()
    wqkvT = nc.dram_tensor("wqkvT", [P, CB, F], F32R, kind="ExternalInput").ap()
    wprojT = nc.dram_tensor("wprojT", [P, CB, C], F32R, kind="ExternalInput").ap()
    bproj = nc.dram_tensor("bproj", [P, CB], F32, kind="ExternalInput").ap()
    maskd = nc.dram_tensor("maskd", [P, NB], F32, kind="ExternalInput").ap()
    cosd = nc.dram_tensor("cosd", [P, NB, D // 2], F32, kind="ExternalInput").ap()
    sind = nc.dram_tensor("sind", [P, NB, D], F32, kind="ExternalInput").ap()
    outT = nc.dram_tensor("outT", [C, N], F32, kind="ExternalOutput").ap()

    with tile.TileContext(nc) as tc:
        with (
            tc.tile_pool(name="singles", bufs=1) as singles,
            tc.tile_pool(name="xblk", bufs=4) as xblk_pool,
            tc.tile_pool(name="rope", bufs=4) as rope_pool,
            tc.tile_pool(name="eT", bufs=4) as eT_pool,
            tc.tile_pool(name="den", bufs=2) as den_pool,
            tc.tile_pool(name="outp", bufs=2) as out_pool,
            tc.tile_pool(name="qT", bufs=2) as qT_pool,
            tc.tile_pool(name="ps", bufs=4, space="PSUM") as ps_pool,
        ):
            # ---- resident inputs ----
            wqkvT_sb = singles.tile([P, CB, F], F32R)
            nc.sync.dma_start(out=wqkvT_sb, in_=wqkvT)
            wprojT_sb = singles.tile([P, CB, C], F32R)
            nc.sync.dma_start(out=wprojT_sb, in_=wprojT)
            bproj_sb = singles.tile([P, CB], F32)
            nc.sync.dma_start(out=bproj_sb, in_=bproj)
            mask_sb = singles.tile([P, NB], F32)
            nc.sync.dma_start(out=mask_sb, in_=maskd)
            cos_sb = singles.tile([P, NB, D // 2], F32)
            nc.sync.dma_start(out=cos_sb, in_=cosd)
            sin_sb = singles.tile([P, NB, D], F32)   # [-sin | +sin]
            nc.sync.dma_start(out=sin_sb, in_=sind)
            ident = singles.tile([P, P], F32)
            make_identity(nc, ident)
            identr_t = singles.tile([P, P], F32R)
            nc.vector.tensor_copy(out=identr_t, in_=ident)
            identr = identr_t[:]

            # ---- resident intermediates ----
            kT_sb = singles.tile([P, PAIRS, N], F32R)
            vaug_sb = singles.tile([P, NB, VW], BF16)
            # ones-columns of vaug (never overwritten by the v copies)
            vaug4 = vaug_sb[:].rearrange("p i (h e) -> p i h e", e=D + 1)
            nc.vector.memset(vaug4[:, :, :, D:D + 1], 1.0)
            # Schraudolph per-key bias: A*mask + B
            bprime = singles.tile([P, NB], F32)
            nc.vector.tensor_scalar(out=bprime, in0=mask_sb[:],
                                    scalar1=A_SCHR, scalar2=B_SCHR,
                                    op0=AluOpType.mult, op1=AluOpType.add)

            def rope_block(ps, i, dst):
                """Rotate [128, 512] from PSUM column-half `ps` into SBUF dst.

                Feature layout per head: [un(32) | ev(32)] (host W permute).
                r = t*[cos|cos] + swap(t)*[-sin|+sin]
                """
                t4 = ps.rearrange("p (h s j) -> p h s j", h=H, s=2)
                cosb = (cos_sb[:, i, :].unsqueeze(1).unsqueeze(2)
                        .broadcast_to([P, H, 2, D // 2]))
                sinb = (sin_sb[:, i, :].rearrange("p (s j) -> p s j", s=2)
                        .unsqueeze(1).broadcast_to([P, H, 2, D // 2]))
                m1 = rope_pool.tile([P, C], F32, tag="ropetmp")
                m2 = rope_pool.tile([P, C], F32, tag="ropetmp")
                m14 = m1[:].rearrange("p (h s j) -> p h s j", h=H, s=2)
                m24 = m2[:].rearrange("p (h s j) -> p h s j", h=H, s=2)
                nc.vector.tensor_tensor(out=m14, in0=t4, in1=cosb,
                                        op=AluOpType.mult)
                nc.vector.tensor_tensor(out=m24, in0=t4[:, :, ::-1, :],
                                        in1=sinb, op=AluOpType.mult)
                # combine on gpsimd (SBUF-only op) to unload DVE
                nc.gpsimd.tensor_tensor(out=dst[:], in0=m1[:], in1=m2[:],
                                        op=AluOpType.add)

            def qk_mm(i, col0):
                """Project token block i; returns (psum tile, xblk tile).
                qkv lands in u[:, 0:512]; u[:, 512:1024] is transpose scratch."""
                xblk = xblk_pool.tile([P, CB, P], F32R, tag="xblk")
                nc.sync.dma_start(out=xblk, in_=xT[i])
                u = ps_pool.tile([P, 1024], F32, tag="u")
                for cc in range(CB):
                    nc.tensor.matmul(u[:, 0:512], lhsT=xblk[:, cc, :],
                                     rhs=wqkvT_sb[:, cc, col0:col0 + C],
                                     start=(cc == 0), stop=(cc == CB - 1))
                return u, xblk

            def qk_finish(u, i, dstT, dst_nsl):
                """rope + transpose + evacuate u into dstT[:, :, dst_nsl]."""
                rot = rope_pool.tile([P, C], F32R, tag="qk")
                rope_block(u[:, 0:512], i, rot)
                ur = u[:].bitcast(F32R)
                for j in range(PAIRS):
                    nc.tensor.transpose(ur[:, 512 + j * P:512 + (j + 1) * P],
                                        rot[:, bass.ts(j, P)], identr)
                nc.vector.tensor_copy(
                    out=dstT[:, :, dst_nsl],
                    in_=ur[:, 512:1024].rearrange("p (j n) -> p j n", j=PAIRS))

            def v_mm(uv, half, i, xblk):
                for cc in range(CB):
                    nc.tensor.matmul(uv[:, half * 512:half * 512 + 512],
                                     lhsT=xblk[:, cc, :],
                                     rhs=wqkvT_sb[:, cc, 1024:1536],
                                     start=(cc == 0), stop=(cc == CB - 1))

            def a_block_pair(i0, dstT, nb0, with_v):
                """Blocks i0, i0+1: matmuls first, transposes after (keeps the
                in-order PE queue from stalling on the DVE rope)."""
                i1 = i0 + 1
                u0, x0 = qk_mm(i0, 512 if with_v else 0)
                u1, x1 = qk_mm(i1, 512 if with_v else 0)
                if with_v:
                    uv = ps_pool.tile([P, 1024], F32, tag="u")
                    v_mm(uv, 0, i0, x0)
                    v_mm(uv, 1, i1, x1)
                qk_finish(u0, i0, dstT, bass.ts(nb0, P))
                qk_finish(u1, i1, dstT, bass.ts(nb0 + 1, P))
                if with_v:
                    nc.vector.tensor_copy(
                        out=vaug4[:, i0:i0 + 2, :, 0:D],
                        in_=uv[:].rearrange("p (b h d) -> p b h d", b=2, h=H))

            def body():
                # ---- stage A: k + v for all token blocks ----
                for i2 in range(NB // 2):
                    i0 = 2 * i2
                    a_block_pair(i0, kT_sb, i0, True)

                if "no_b" in ablate:
                    # keep-alive dumps so the compiler can not DCE stage A
                    nc.sync.dma_start(out=outT[0:P, :],
                                      in_=kT_sb[:, 0, :].bitcast(F32))
                    vv = singles.tile([P, NB, 128], F32)
                    nc.vector.tensor_copy(out=vv, in_=vaug_sb[:, :, 0:128])
                    nc.sync.dma_start(
                        out=outT[P:2 * P, :],
                        in_=vv[:].rearrange("p i e -> p (i e)"))
                    if "a_full" in ablate:
                        for ncq in range(NCH):
                            qT_sb = qT_pool.tile([P, PAIRS, 512], F32R, tag="qT")
                            for ib2 in range(2):
                                a_block_pair(ncq * 4 + 2 * ib2, qT_sb,
                                             2 * ib2, False)
                            nc.sync.dma_start(
                                out=outT[2 * P:3 * P, bass.ts(ncq, 512)],
                                in_=qT_sb[:, 0, :].bitcast(F32))
                    return

                for ncq in range(NCH):
                    qsl = bass.ts(ncq, 512)
                    # ---- q for this chunk (double-buffered tile) ----
                    qT_sb = qT_pool.tile([P, PAIRS, 512], F32R, tag="qT")
                    for ib2 in range(2):
                        a_block_pair(ncq * 4 + 2 * ib2, qT_sb,
                                     2 * ib2, False)
                    # ---- stage B ----
                    attn_t = out_pool.tile([P, CB, 512], F32R, tag="attn")
                    for pj in range(PAIRS):
                        hA, hB = 2 * pj, 2 * pj + 1
                        num = ps_pool.tile([P, 1024], F32, tag="u")

                        def av(mb, eT):
                            nc.tensor.matmul(
                                num[0:D + 1, 0:512],
                                lhsT=vaug_sb[:, mb, hA * 65:(hA + 1) * 65],
                                rhs=eT[:, 0:512],
                                start=(mb == 0), stop=(mb == NB - 1))
                            nc.tensor.matmul(
                                num[0:D + 1, 512:1024],
                                lhsT=vaug_sb[:, mb, hB * 65:(hB + 1) * 65],
                                rhs=eT[:, 512:1024],
                                start=(mb == 0), stop=(mb == NB - 1))

                        pend = []
                        for mb in range(NB):
                            pairt = ps_pool.tile([P, 1024], F32, tag="u")
                            nc.tensor.matmul(pairt[:, 0:512],
                                             lhsT=kT_sb[0:64, pj, bass.ts(mb, P)],
                                             rhs=qT_sb[0:64, pj, :],
                                             start=True, stop=True,
                                             tile_position=(0, 0))
                            nc.tensor.matmul(pairt[:, 512:1024],
                                             lhsT=kT_sb[64:128, pj, bass.ts(mb, P)],
                                             rhs=qT_sb[64:128, pj, :],
                                             start=True, stop=True,
                                             tile_position=(64, 0))
                            eT = eT_pool.tile([P, 1024], BF16, tag="eT")
                            if mb % 4 == 3 and s_act < NB:
                                # Schraudolph exp: bitcast(int16(z + A*mask+B))
                                # on DVE (gpsimd cannot read PSUM)
                                nc.vector.tensor_scalar(
                                    out=eT[:].bitcast(I16), in0=pairt[:],
                                    scalar1=bprime[:, mb:mb + 1],
                                    scalar2=None, op0=AluOpType.add)
                            else:
                                nc.scalar.activation(
                                    out=eT[:], in_=pairt[:], func=EXP,
                                    bias=mask_sb[:, mb:mb + 1],
                                    scale=1.0 / A_SCHR)
                            pend.append((mb, eT))
                            # software pipeline: AV lags scores by 2 so the
                            # in-order PE queue never waits on exp
                            if len(pend) > 2:
                                av(*pend.pop(0))
                        for it in pend:
                            av(*it)
                        rcA = den_pool.tile([1, 512], F32, tag="recip")
                        rcB = den_pool.tile([1, 512], F32, tag="recip")
                        denA = den_pool.tile([64, 512], F32, tag="den")
                        denB = den_pool.tile([64, 512], F32, tag="den")
                        nc.vector.reciprocal(out=rcA, in_=num[64:65, 0:512])
                        nc.vector.reciprocal(out=rcB, in_=num[64:65, 512:1024])
                        nc.gpsimd.partition_broadcast(denA[:], rcA[:])
                        nc.gpsimd.partition_broadcast(denB[:], rcB[:])
                        nc.vector.tensor_tensor(out=attn_t[0:64, pj, :],
                                                in0=num[0:64, 0:512],
                                                in1=denA[:],
                                                op=AluOpType.mult)
                        nc.vector.tensor_tensor(out=attn_t[64:128, pj, :],
                                                in0=num[0:64, 512:1024],
                                                in1=denB[:],
                                                op=AluOpType.mult)
                    # ---- stage C ----
                    if "no_c" in ablate:
                        nc.sync.dma_start(out=outT[0:P, qsl],
                                          in_=attn_t[:, 0, :].bitcast(F32))
                        continue
                    for ob2 in range(CB // 2):
                        uo = ps_pool.tile([P, 1024], F32, tag="u")
                        for half in range(2):
                            ob = 2 * ob2 + half
                            for cc in range(CB):
                                nc.tensor.matmul(
                                    uo[:, half * 512:half * 512 + 512],
                                    lhsT=wprojT_sb[:, cc, bass.ts(ob, P)],
                                    rhs=attn_t[:, cc, :],
                                    start=(cc == 0), stop=(cc == CB - 1))
                        for half in range(2):
                            ob = 2 * ob2 + half
                            ot = out_pool.tile([P, 512], F32, tag="out")
                            nc.vector.tensor_scalar(
                                out=ot[:], in0=uo[:, half * 512:half * 512 + 512],
                                scalar1=bproj_sb[:, ob:ob + 1],
                                scalar2=None, op0=AluOpType.add)
                            nc.sync.dma_start(out=outT[bass.ts(ob, P), qsl],
                                              in_=ot[:])

            if repeats > 1:
                with tc.For_i(0, repeats) as _i:
                    body()
            else:
                body()

    nc.compile()
    return nc


def prep_in_maps(x, mask, W_qkv, W_proj, b_proj, num_cls_token):
    x = np.asarray(x, dtype=np.float32)
    mask = np.asarray(mask, dtype=np.float32)
    W_qkv = np.asarray(W_qkv, dtype=np.float32)
    W_proj = np.asarray(W_proj, dtype=np.float32)
    b_proj = np.asarray(b_proj, dtype=np.float32)
    ncls = int(np.asarray(num_cls_token))

    # De-interleave q/k head dims: within each head, rows [0,2,..,62, 1,3,..,63]
    # so rope operates on contiguous [un|ev] halves. Scores are invariant to
    # a shared q/k permutation. Scale k by A*0.125 for the Schraudolph path.
    perm64 = np.concatenate([np.arange(0, D, 2), np.arange(1, D, 2)])
    perm = np.arange(F)
    for sec in range(2):                       # q rows 0:512, k rows 512:1024
        for h in range(H):
            base = sec * C + h * D
            perm[base:base + D] = base + perm64
    Wp = W_qkv[perm]
    Wp = Wp.copy()
    Wp[C:2 * C] *= np.float32(A_SCHR * 0.125)   # fold A*scale into k
    wqkvT = np.ascontiguousarray(Wp.T)
    wprojT = np.ascontiguousarray(W_proj.T)

    # rope tables in de-interleaved order: j-th column is original dim 2j
    cos = np.ones((N, D // 2), dtype=np.float32)
    sin = np.zeros((N, D // 2), dtype=np.float32)
    if ncls < N:
        inv_freq = (1.0 / (10000.0 ** (np.arange(0, D, 2, dtype=np.float32)
                                       / np.float32(D)))).astype(np.float32)
        pos = np.arange(N - ncls, dtype=np.float32)
        freqs = pos[:, None] * inv_freq[None, :]
        cos[ncls:] = np.cos(freqs).astype(np.float32)
        sin[ncls:] = np.sin(freqs).astype(np.float32)
    sin_ext = np.concatenate([-sin, sin], axis=1)           # (N, 64)

    def part_major(a):
        return np.ascontiguousarray(
            a.reshape(CB, P, *a.shape[1:]).transpose(
                1, 0, *range(2, a.ndim + 1)))

    def tok_major(a):
        return np.ascontiguousarray(
            a.reshape(NB, P, *a.shape[1:]).transpose(
                1, 0, *range(2, a.ndim + 1)))

    wqkvT_h = part_major(wqkvT)
    wprojT_h = part_major(wprojT)
    bproj_h = part_major(b_proj)
    cos_h = tok_major(cos)
    sin_h = tok_major(sin_ext)

    in_maps = []
    for b in range(B):
        xb = x[b].reshape(NB, P, CB, P).transpose(0, 3, 2, 1)
        in_maps.append({
            "xT": np.ascontiguousarray(xb),
            "wqkvT": wqkvT_h,
            "wprojT": wprojT_h,
            "bproj": bproj_h,
            "maskd": tok_major(mask[b]),
            "cosd": cos_h,
            "sind": sin_h,
        })
    return in_maps


def gather_out(results):
    out = np.empty((B, N, C), dtype=np.float32)
    for b in range(B):
        out[b] = results[b]["outT"].T
    return out


_NC_CACHE = {}

TRACE = False          # test.py sets True to capture NTFF profile + exec time
LAST_RES = None        # BassKernelResults of the last kernel() call


def kernel(**inputs) -> np.ndarray:
    global LAST_RES
    from concourse.bass_utils import run_bass_kernel_spmd
    key = "single"
    if key not in _NC_CACHE:
        _NC_CACHE[key] = build_nc(repeats=1)
    nc = _NC_CACHE[key]
    in_maps = prep_in_maps(**inputs)
    res = run_bass_kernel_spmd(nc, in_maps, list(range(B)), trace=TRACE)
    LAST_RES = res
    return gather_out(res.results)


# revision 25
# speedup vs baseline: 1.4939x; 1.4939x over previous
"""AttentionWithRotaryPositionalEmbedding — Trainium2 Bass kernel (v2).

Shapes (hardcoded, from the problem spec):
  x: (8, 2048, 512), mask: (8, 2048), W_qkv: (1536, 512),
  W_proj: (512, 512), b_proj: (512,), num_cls_token: scalar
Sharding: data-parallel over batch B=8 across the 8 NeuronCores; weights
replicated. No collectives.

Per-core dataflow (batch b):
  Stage A (k/v):  k = x_b @ W_k^T via PE (W_k host-permuted so each head is
    [un(32)|ev(32)] and pre-scaled by 2^23/ln2 * 0.125 for the softmax
    bit-trick); rope as 3 full-width DVE ops (mult, mult-on-swapped-view,
    add); PE transpose to kT[d, n]; v copied into an augmented [v|1] tile
    whose ones-column yields softmax denominators from the AV matmul.
  Per query chunk (512): same for q, then attention:
    scoresT[m, n] = k^T q    PE, two heads packed via row tiles (0,0)/(64,0)
    eT = exp(..)             key blocks 0..9: ScalarE exp (scale=1/A,
                             bias=mask); blocks 10..15: GPSIMD Schraudolph
                             (int32 add of A*mask+B, bitcast to fp32)
    num[d|den, n] += vaug^T eT   PE accumulate
    attnT = num * (1/den)    DVE reciprocal + gpsimd partition_broadcast
  outT = W_proj attnT + b_proj   PE + DVE bias (stored transposed; host
                                 transposes back)
"""

import numpy as np

import concourse.bass as bass
import concourse.tile as tile
from concourse import bacc, mybir
from concourse.alu_op_type import AluOpType
from concourse.masks import make_identity

P = 128
B = 8
N = 2048
C = 512
H = 8
D = 64
F = 3 * C          # 1536
NB = N // P        # 16 token blocks
CB = C // P        # 4 contraction chunks
PAIRS = H // 2     # 4 head pairs
NCH = N // 512     # 4 query chunks of 512
VW = H * (D + 1)   # 520

A_SCHR = float(2 ** 7) / float(np.log(2.0))    # bf16-space Schraudolph slope
C_SCHR = 366393.0 / 65536.0
B_SCHR = 127.0 * 2 ** 7 - C_SCHR
S_ACT = 12          # key blocks [0, S_ACT) use ScalarE exp; rest DVE Schraudolph

F32 = mybir.dt.float32
F32R = mybir.dt.float32r
BF16 = mybir.dt.bfloat16
I16 = mybir.dt.int16
EXP = mybir.ActivationFunctionType.Exp


def build_nc(repeats: int = 1, debug: bool = False, ablate: frozenset = frozenset()):
    """ablate: 'no_schr' -> all-ScalarE exp (error attribution);
    'no_b' / 'no_c' -> skip stages (timing only)."""
    s_act = NB if "no_schr" in ablate else S_ACT
    nc = bacc.Bacc("TRN2", target_bir_lowering=False, debug=False, num_devices=B)

    xT = nc.dram_tensor("xT", [NB, P, CB, P], F32R, kind="ExternalInput").ap()
    wqkvT = nc.dram_tensor("wqkvT", [P, CB, F], F32R, kind="ExternalInput").ap()
    wprojT = nc.dram_tensor("wprojT", [P, CB, C], F32R, kind="ExternalInput").ap()
    bproj = nc.dram_tensor("bproj", [P, CB], F32, kind="ExternalInput").ap()
    maskd = nc.dram_tensor("maskd", [P, NB], F32, kind="ExternalInput").ap()
    cosd = nc.dram_tensor("cosd", [P, NB, D // 2], F32, kind="ExternalInput").ap()
    sind = nc.dram_tensor("sind", [P, NB, D], F32, kind="ExternalInput").ap()
    outT = nc.dram_tensor("outT", [C, N], F32, kind="ExternalOutput").ap()

    with tile.TileContext(nc) as tc:
        with (
            tc.tile_pool(name="singles", bufs=1) as singles,
            tc.tile_pool(name="xblk", bufs=4) as xblk_pool,
            tc.tile_pool(name="rope", bufs=4) as rope_pool,
            tc.tile_pool(name="eT", bufs=4) as eT_pool,
            tc.tile_pool(name="den", bufs=2) as den_pool,
            tc.tile_pool(name="outp", bufs=2) as out_pool,
            tc.tile_pool(name="qT", bufs=2) as qT_pool,
            tc.tile_pool(name="pair", bufs=2, space="PSUM") as pair_ps,
            tc.tile_pool(name="num", bufs=2, space="PSUM") as num_pool,
        ):
            # ---- resident inputs ----
            wqkvT_sb = singles.tile([P, CB, F], F32R)
            nc.sync.dma_start(out=wqkvT_sb, in_=wqkvT)
            wprojT_sb = singles.tile([P, CB, C], F32R)
            nc.sync.dma_start(out=wprojT_sb, in_=wprojT)
            bproj_sb = singles.tile([P, CB], F32)
            nc.sync.dma_start(out=bproj_sb, in_=bproj)
            mask_sb = singles.tile([P, NB], F32)
            nc.sync.dma_start(out=mask_sb, in_=maskd)
            cos_sb = singles.tile([P, NB, D // 2], F32)
            nc.sync.dma_start(out=cos_sb, in_=cosd)
            sin_sb = singles.tile([P, NB, D], F32)   # [-sin | +sin]
            nc.sync.dma_start(out=sin_sb, in_=sind)
            ident = singles.tile([P, P], F32)
            make_identity(nc, ident)
            identr_t = singles.tile([P, P], F32R)
            nc.vector.tensor_copy(out=identr_t, in_=ident)
            identr = identr_t[:]

            # ---- resident intermediates ----
            kT_sb = singles.tile([P, PAIRS, N], F32R)
            vaug_sb = singles.tile([P, NB, VW], BF16)
            # ones-columns of vaug (never overwritten by the v copies)
            vaug4 = vaug_sb[:].rearrange("p i (h e) -> p i h e", e=D + 1)
            nc.vector.memset(vaug4[:, :, :, D:D + 1], 1.0)
            # Schraudolph per-key bias: A*mask + B
            bprime = singles.tile([P, NB], F32)
            nc.vector.tensor_scalar(out=bprime, in0=mask_sb[:],
                                    scalar1=A_SCHR, scalar2=B_SCHR,
                                    op0=AluOpType.mult, op1=AluOpType.add)

            def rope_block(ps, i, dst):
                """Rotate [128, 512] from PSUM column-half `ps` into SBUF dst.

                Feature layout per head: [un(32) | ev(32)] (host W permute).
                r = t*[cos|cos] + swap(t)*[-sin|+sin]
                """
                t4 = ps.rearrange("p (h s j) -> p h s j", h=H, s=2)
                cosb = (cos_sb[:, i, :].unsqueeze(1).unsqueeze(2)
                        .broadcast_to([P, H, 2, D // 2]))
                sinb = (sin_sb[:, i, :].rearrange("p (s j) -> p s j", s=2)
                        .unsqueeze(1).broadcast_to([P, H, 2, D // 2]))
                m1 = rope_pool.tile([P, C], F32, tag="ropetmp")
                m2 = rope_pool.tile([P, C], F32, tag="ropetmp")
                m14 = m1[:].rearrange("p (h s j) -> p h s j", h=H, s=2)
                m24 = m2[:].rearrange("p (h s j) -> p h s j", h=H, s=2)
                nc.vector.tensor_tensor(out=m14, in0=t4, in1=cosb,
                                        op=AluOpType.mult)
                nc.vector.tensor_tensor(out=m24, in0=t4[:, :, ::-1, :],
                                        in1=sinb, op=AluOpType.mult)
                # combine on gpsimd (SBUF-only op) to unload DVE
                nc.gpsimd.tensor_tensor(out=dst[:], in0=m1[:], in1=m2[:],
                                        op=AluOpType.add)

            def qk_mm(i, col0):
                """Project token block i; returns (psum tile, xblk tile).
                qkv lands in u[:, 0:512]; u[:, 512:1024] is transpose scratch."""
                xblk = xblk_pool.tile([P, CB, P], F32R, tag="xblk")
                nc.sync.dma_start(out=xblk, in_=xT[i])
                u = pair_ps.tile([P, 1024], F32, tag="pair")
                for cc in range(CB):
                    nc.tensor.matmul(u[:, 0:512], lhsT=xblk[:, cc, :],
                                     rhs=wqkvT_sb[:, cc, col0:col0 + C],
                                     start=(cc == 0), stop=(cc == CB - 1))
                return u, xblk

            def qk_finish(u, i, dstT, dst_nsl):
                """rope + transpose + evacuate u into dstT[:, :, dst_nsl]."""
                rot = rope_pool.tile([P, C], F32R, tag="qk")
                rope_block(u[:, 0:512], i, rot)
                ur = u[:].bitcast(F32R)
                for j in range(PAIRS):
                    nc.tensor.transpose(ur[:, 512 + j * P:512 + (j + 1) * P],
                                        rot[:, bass.ts(j, P)], identr)
                nc.vector.tensor_copy(
                    out=dstT[:, :, dst_nsl],
                    in_=ur[:, 512:1024].rearrange("p (j n) -> p j n", j=PAIRS))

            def v_mm(uv, half, i, xblk):
                for cc in range(CB):
                    nc.tensor.matmul(uv[:, half * 512:half * 512 + 512],
                                     lhsT=xblk[:, cc, :],
                                     rhs=wqkvT_sb[:, cc, 1024:1536],
                                     start=(cc == 0), stop=(cc == CB - 1))

            def a_block_pair(i0, dstT, nb0, with_v):
                """Blocks i0, i0+1: matmuls first, transposes after (keeps the
                in-order PE queue from stalling on the DVE rope)."""
                i1 = i0 + 1
                u0, x0 = qk_mm(i0, 512 if with_v else 0)
                u1, x1 = qk_mm(i1, 512 if with_v else 0)
                if with_v:
                    uv = num_pool.tile([P, 1024], F32, tag="num")
                    v_mm(uv, 0, i0, x0)
                    v_mm(uv, 1, i1, x1)
                qk_finish(u0, i0, dstT, bass.ts(nb0, P))
                qk_finish(u1, i1, dstT, bass.ts(nb0 + 1, P))
                if with_v:
                    nc.vector.tensor_copy(
                        out=vaug4[:, i0:i0 + 2, :, 0:D],
                        in_=uv[:].rearrange("p (b h d) -> p b h d", b=2, h=H))

            def body():
                # ---- stage A: k + v for all token blocks ----
                for i2 in range(NB // 2):
                    i0 = 2 * i2
                    a_block_pair(i0, kT_sb, i0, True)

                if "no_b" in ablate:
                    # keep-alive dumps so the compiler can not DCE stage A
                    nc.sync.dma_start(out=outT[0:P, :],
                                      in_=kT_sb[:, 0, :].bitcast(F32))
                    vv = singles.tile([P, NB, 128], F32)
                    nc.vector.tensor_copy(out=vv, in_=vaug_sb[:, :, 0:128])
                    nc.sync.dma_start(
                        out=outT[P:2 * P, :],
                        in_=vv[:].rearrange("p i e -> p (i e)"))
                    return

                # ---- two passes over query-chunk pairs (2 x 512 queries) ----
                for cp in range(2):
                    # q for the 8 token blocks of this chunk pair
                    qT_sb = qT_pool.tile([P, PAIRS, 1024], F32R, tag="qT")
                    for j in range(4):
                        a_block_pair(cp * 8 + 2 * j, qT_sb, 2 * j, False)

                    attn0 = out_pool.tile([P, CB, 512], F32R, tag="attn")
                    attn1 = out_pool.tile([P, CB, 512], F32R, tag="attn")
                    for pj in range(PAIRS):
                        hA, hB = 2 * pj, 2 * pj + 1
                        num_A = num_pool.tile([P, 1024], F32, tag="num")
                        num_B = num_pool.tile([P, 1024], F32, tag="num")

                        def av(mb, eT0, eT1):
                            for h, numt, off in ((hA, num_A, 0), (hB, num_B, 512)):
                                for ci, ee in ((0, eT0), (1, eT1)):
                                    nc.tensor.matmul(
                                        numt[0:D + 1, ci * 512:ci * 512 + 512],
                                        lhsT=vaug_sb[:, mb, h * 65:(h + 1) * 65],
                                        rhs=ee[:, off:off + 512],
                                        start=(mb == 0), stop=(mb == NB - 1))

                        pend = []
                        for mb in range(NB):
                            p0 = pair_ps.tile([P, 1024], F32, tag="pair")
                            p1 = pair_ps.tile([P, 1024], F32, tag="pair")
                            for ci, pt in ((0, p0), (1, p1)):
                                nc.tensor.matmul(
                                    pt[:, 0:512],
                                    lhsT=kT_sb[0:64, pj, bass.ts(mb, P)],
                                    rhs=qT_sb[0:64, pj, ci * 512:ci * 512 + 512],
                                    start=True, stop=True,
                                    tile_position=(0, 0))
                                nc.tensor.matmul(
                                    pt[:, 512:1024],
                                    lhsT=kT_sb[64:128, pj, bass.ts(mb, P)],
                                    rhs=qT_sb[64:128, pj, ci * 512:ci * 512 + 512],
                                    start=True, stop=True,
                                    tile_position=(64, 0))
                            eT0 = eT_pool.tile([P, 1024], BF16, tag="eT")
                            eT1 = eT_pool.tile([P, 1024], BF16, tag="eT")
                            use_dve = (mb % 5 == 4) and s_act < NB
                            for pt, ee in ((p0, eT0), (p1, eT1)):
                                if use_dve:
                                    # Schraudolph exp on DVE:
                                    # bitcast(int16(z + A*mask+B))
                                    nc.vector.tensor_scalar(
                                        out=ee[:].bitcast(I16), in0=pt[:],
                                        scalar1=bprime[:, mb:mb + 1],
                                        scalar2=None, op0=AluOpType.add)
                                else:
                                    nc.scalar.activation(
                                        out=ee[:], in_=pt[:], func=EXP,
                                        bias=mask_sb[:, mb:mb + 1],
                                        scale=1.0 / A_SCHR)
                            pend.append((mb, eT0, eT1))
                            if len(pend) > 1:
                                av(*pend.pop(0))
                        for it in pend:
                            av(*it)
                        # normalize: row 64 of num = denominators
                        rcA = den_pool.tile([1, 1024], F32, tag="recip")
                        rcB = den_pool.tile([1, 1024], F32, tag="recip")
                        denA = den_pool.tile([64, 1024], F32, tag="den")
                        denB = den_pool.tile([64, 1024], F32, tag="den")
                        nc.vector.reciprocal(out=rcA, in_=num_A[64:65, :])
                        nc.vector.reciprocal(out=rcB, in_=num_B[64:65, :])
                        nc.gpsimd.partition_broadcast(denA[:], rcA[:])
                        nc.gpsimd.partition_broadcast(denB[:], rcB[:])
                        for ci, at in ((0, attn0), (1, attn1)):
                            nc.vector.tensor_tensor(
                                out=at[0:64, pj, :],
                                in0=num_A[0:64, ci * 512:ci * 512 + 512],
                                in1=denA[:, ci * 512:ci * 512 + 512],
                                op=AluOpType.mult)
                            nc.vector.tensor_tensor(
                                out=at[64:128, pj, :],
                                in0=num_B[0:64, ci * 512:ci * 512 + 512],
                                in1=denB[:, ci * 512:ci * 512 + 512],
                                op=AluOpType.mult)
                    # ---- stage C for both chunks of this pass ----
                    for ci, at in ((0, attn0), (1, attn1)):
                        qsl = bass.ts(2 * cp + ci, 512)
                        if "no_c" in ablate:
                            nc.sync.dma_start(out=outT[0:P, qsl],
                                              in_=at[:, 0, :].bitcast(F32))
                            continue
                        for ob2 in range(CB // 2):
                            uo = pair_ps.tile([P, 1024], F32, tag="pair")
                            for half in range(2):
                                ob = 2 * ob2 + half
                                for cc in range(CB):
                                    nc.tensor.matmul(
                                        uo[:, half * 512:half * 512 + 512],
                                        lhsT=wprojT_sb[:, cc, bass.ts(ob, P)],
                                        rhs=at[:, cc, :],
                                        start=(cc == 0), stop=(cc == CB - 1))
                            for half in range(2):
                                ob = 2 * ob2 + half
                                ot = out_pool.tile([P, 512], F32, tag="out")
                                nc.vector.tensor_scalar(
                                    out=ot[:],
                                    in0=uo[:, half * 512:half * 512 + 512],
                                    scalar1=bproj_sb[:, ob:ob + 1],
                                    scalar2=None, op0=AluOpType.add)
                                nc.sync.dma_start(out=outT[bass.ts(ob, P), qsl],
                                                  in_=ot[:])

            if repeats > 1:
                with tc.For_i(0, repeats) as _i:
                    body()
            else:
                body()

    nc.compile()
    return nc


def prep_in_maps(x, mask, W_qkv, W_proj, b_proj, num_cls_token):
    x = np.asarray(x, dtype=np.float32)
    mask = np.asarray(mask, dtype=np.float32)
    W_qkv = np.asarray(W_qkv, dtype=np.float32)
    W_proj = np.asarray(W_proj, dtype=np.float32)
    b_proj = np.asarray(b_proj, dtype=np.float32)
    ncls = int(np.asarray(num_cls_token))

    # De-interleave q/k head dims: within each head, rows [0,2,..,62, 1,3,..,63]
    # so rope operates on contiguous [un|ev] halves. Scores are invariant to
    # a shared q/k permutation. Scale k by A*0.125 for the Schraudolph path.
    perm64 = np.concatenate([np.arange(0, D, 2), np.arange(1, D, 2)])
    perm = np.arange(F)
    for sec in range(2):                       # q rows 0:512, k rows 512:1024
        for h in range(H):
            base = sec * C + h * D
            perm[base:base + D] = base + perm64
    Wp = W_qkv[perm]
    Wp = Wp.copy()
    Wp[C:2 * C] *= np.float32(A_SCHR * 0.125)   # fold A*scale into k
    wqkvT = np.ascontiguousarray(Wp.T)
    wprojT = np.ascontiguousarray(W_proj.T)

    # rope tables in de-interleaved order: j-th column is original dim 2j
    cos = np.ones((N, D // 2), dtype=np.float32)
    sin = np.zeros((N, D // 2), dtype=np.float32)
    if ncls < N:
        inv_freq = (1.0 / (10000.0 ** (np.arange(0, D, 2, dtype=np.float32)
                                       / np.float32(D)))).astype(np.float32)
        pos = np.arange(N - ncls, dtype=np.float32)
        freqs = pos[:, None] * inv_freq[None, :]
        cos[ncls:] = np.cos(freqs).astype(np.float32)
        sin[ncls:] = np.sin(freqs).astype(np.float32)
    sin_ext = np.concatenate([-sin, sin], axis=1)           # (N, 64)

    def part_major(a):
        return np.ascontiguousarray(
            a.reshape(CB, P, *a.shape[1:]).transpose(
                1, 0, *range(2, a.ndim + 1)))

    def tok_major(a):
        return np.ascontiguousarray(
            a.reshape(NB, P, *a.shape[1:]).transpose(
                1, 0, *range(2, a.ndim + 1)))

    wqkvT_h = part_major(wqkvT)
    wprojT_h = part_major(wprojT)
    bproj_h = part_major(b_proj)
    cos_h = tok_major(cos)
    sin_h = tok_major(sin_ext)

    in_maps = []
    for b in range(B):
        xb = x[b].reshape(NB, P, CB, P).transpose(0, 3, 2, 1)
        in_maps.append({
            "xT": np.ascontiguousarray(xb),
            "wqkvT": wqkvT_h,
            "wprojT": wprojT_h,
            "bproj": bproj_h,
            "maskd": tok_major(mask[b]),
            "cosd": cos_h,
            "sind": sin_h,
        })
    return in_maps


def gather_out(results):
    out = np.empty((B, N, C), dtype=np.float32)
    for b in range(B):
        out[b] = results[b]["outT"].T
    return out


_NC_CACHE = {}

TRACE = False          # test.py sets True to capture NTFF profile + exec time
LAST_RES = None        # BassKernelResults of the last kernel() call


def kernel(**inputs) -> np.ndarray:
    global LAST_RES
    from concourse.bass_utils import run_bass_kernel_spmd
    key = "single"
    if key not in _NC_CACHE:
        _NC_CACHE[key] = build_nc(repeats=1)
    nc = _NC_CACHE[key]
    in_maps = prep_in_maps(**inputs)
    res = run_bass_kernel_spmd(nc, in_maps, list(range(B)), trace=TRACE)
    LAST_RES = res
    return gather_out(res.results)


# revision 26
# speedup vs baseline: 1.6865x; 1.1289x over previous
"""AttentionWithRotaryPositionalEmbedding — Trainium2 Bass kernel (v2).

Shapes (hardcoded, from the problem spec):
  x: (8, 2048, 512), mask: (8, 2048), W_qkv: (1536, 512),
  W_proj: (512, 512), b_proj: (512,), num_cls_token: scalar
Sharding: data-parallel over batch B=8 across the 8 NeuronCores; weights
replicated. No collectives.

Per-core dataflow (batch b):
  Stage A (k/v):  k = x_b @ W_k^T via PE (W_k host-permuted so each head is
    [un(32)|ev(32)] and pre-scaled by 2^23/ln2 * 0.125 for the softmax
    bit-trick); rope as 3 full-width DVE ops (mult, mult-on-swapped-view,
    add); PE transpose to kT[d, n]; v copied into an augmented [v|1] tile
    whose ones-column yields softmax denominators from the AV matmul.
  Per query chunk (512): same for q, then attention:
    scoresT[m, n] = k^T q    PE, two heads packed via row tiles (0,0)/(64,0)
    eT = exp(..)             key blocks 0..9: ScalarE exp (scale=1/A,
                             bias=mask); blocks 10..15: GPSIMD Schraudolph
                             (int32 add of A*mask+B, bitcast to fp32)
    num[d|den, n] += vaug^T eT   PE accumulate
    attnT = num * (1/den)    DVE reciprocal + gpsimd partition_broadcast
  outT = W_proj attnT + b_proj   PE + DVE bias (stored transposed; host
                                 transposes back)
"""

import numpy as np

import concourse.bass as bass
import concourse.tile as tile
from concourse import bacc, mybir
from concourse.alu_op_type import AluOpType
from concourse.masks import make_identity

P = 128
B = 8
N = 2048
C = 512
H = 8
D = 64
F = 3 * C          # 1536
NB = N // P        # 16 token blocks
CB = C // P        # 4 contraction chunks
PAIRS = H // 2     # 4 head pairs
NCH = N // 512     # 4 query chunks of 512
VW = H * (D + 1)   # 520

A_SCHR = float(2 ** 7) / float(np.log(2.0))    # bf16-space Schraudolph slope
C_SCHR = 366393.0 / 65536.0
B_SCHR = 127.0 * 2 ** 7 - C_SCHR
S_ACT = 12          # key blocks [0, S_ACT) use ScalarE exp; rest DVE Schraudolph

F32 = mybir.dt.float32
F32R = mybir.dt.float32r
BF16 = mybir.dt.bfloat16
I16 = mybir.dt.int16
EXP = mybir.ActivationFunctionType.Exp


def build_nc(repeats: int = 1, debug: bool = False, ablate: frozenset = frozenset()):
    """ablate: 'no_schr' -> all-ScalarE exp (error attribution);
    'no_b' / 'no_c' -> skip stages (timing only)."""
    s_act = NB if "no_schr" in ablate else S_ACT
    nc = bacc.Bacc("TRN2", target_bir_lowering=False, debug=False, num_devices=B)

    xT = nc.dram_tensor("xT", [NB, P, CB, P], F32R, kind="ExternalInput").ap()
    wqkvT = nc.dram_tensor("wqkvT", [P, CB, F], F32R, kind="ExternalInput").ap()
    wprojT = nc.dram_tensor("wprojT", [P, CB, C], F32R, kind="ExternalInput").ap()
    bproj = nc.dram_tensor("bproj", [P, CB], F32, kind="ExternalInput").ap()
    maskd = nc.dram_tensor("maskd", [P, NB], F32, kind="ExternalInput").ap()
    cosd = nc.dram_tensor("cosd", [P, NB, D // 2], F32, kind="ExternalInput").ap()
    sind = nc.dram_tensor("sind", [P, NB, D], F32, kind="ExternalInput").ap()
    outT = nc.dram_tensor("outT", [C, N], F32, kind="ExternalOutput").ap()

    with tile.TileContext(nc) as tc:
        with (
            tc.tile_pool(name="singles", bufs=1) as singles,
            tc.tile_pool(name="xblk", bufs=4) as xblk_pool,
            tc.tile_pool(name="rope", bufs=4) as rope_pool,
            tc.tile_pool(name="eT", bufs=4) as eT_pool,
            tc.tile_pool(name="den", bufs=2) as den_pool,
            tc.tile_pool(name="outp", bufs=2) as out_pool,
            tc.tile_pool(name="qT", bufs=2) as qT_pool,
            tc.tile_pool(name="pair", bufs=2, space="PSUM") as pair_ps,
            tc.tile_pool(name="num", bufs=2, space="PSUM") as num_pool,
        ):
            # ---- resident inputs ----
            wqkvT_sb = singles.tile([P, CB, F], F32R)
            nc.sync.dma_start(out=wqkvT_sb, in_=wqkvT)
            wprojT_sb = singles.tile([P, CB, C], F32R)
            nc.sync.dma_start(out=wprojT_sb, in_=wprojT)
            bproj_sb = singles.tile([P, CB], F32)
            nc.sync.dma_start(out=bproj_sb, in_=bproj)
            mask_sb = singles.tile([P, NB], F32)
            nc.sync.dma_start(out=mask_sb, in_=maskd)
            cos_sb = singles.tile([P, NB, D // 2], F32)
            nc.sync.dma_start(out=cos_sb, in_=cosd)
            sin_sb = singles.tile([P, NB, D], F32)   # [-sin | +sin]
            nc.sync.dma_start(out=sin_sb, in_=sind)
            ident = singles.tile([P, P], F32)
            make_identity(nc, ident)
            identr_t = singles.tile([P, P], F32R)
            nc.vector.tensor_copy(out=identr_t, in_=ident)
            identr = identr_t[:]

            # ---- resident intermediates ----
            kT_sb = singles.tile([P, PAIRS, N], F32R)
            vaug_sb = singles.tile([P, NB, VW], BF16)
            # ones-columns of vaug (never overwritten by the v copies)
            vaug4 = vaug_sb[:].rearrange("p i (h e) -> p i h e", e=D + 1)
            nc.vector.memset(vaug4[:, :, :, D:D + 1], 1.0)
            # Schraudolph per-key bias: A*mask + B
            bprime = singles.tile([P, NB], F32)
            nc.vector.tensor_scalar(out=bprime, in0=mask_sb[:],
                                    scalar1=A_SCHR, scalar2=B_SCHR,
                                    op0=AluOpType.mult, op1=AluOpType.add)

            def rope_block(ps, i, dst):
                """Rotate [128, 512] from PSUM column-half `ps` into SBUF dst.

                Feature layout per head: [un(32) | ev(32)] (host W permute).
                r = t*[cos|cos] + swap(t)*[-sin|+sin]
                """
                t4 = ps.rearrange("p (h s j) -> p h s j", h=H, s=2)
                cosb = (cos_sb[:, i, :].unsqueeze(1).unsqueeze(2)
                        .broadcast_to([P, H, 2, D // 2]))
                sinb = (sin_sb[:, i, :].rearrange("p (s j) -> p s j", s=2)
                        .unsqueeze(1).broadcast_to([P, H, 2, D // 2]))
                m1 = rope_pool.tile([P, C], F32, tag="ropetmp")
                m2 = rope_pool.tile([P, C], F32, tag="ropetmp")
                m14 = m1[:].rearrange("p (h s j) -> p h s j", h=H, s=2)
                m24 = m2[:].rearrange("p (h s j) -> p h s j", h=H, s=2)
                nc.vector.tensor_tensor(out=m14, in0=t4, in1=cosb,
                                        op=AluOpType.mult)
                nc.vector.tensor_tensor(out=m24, in0=t4[:, :, ::-1, :],
                                        in1=sinb, op=AluOpType.mult)
                # combine on gpsimd (SBUF-only op) to unload DVE
                nc.gpsimd.tensor_tensor(out=dst[:], in0=m1[:], in1=m2[:],
                                        op=AluOpType.add)

            def qk_mm(i, col0):
                """Project token block i; returns (psum tile, xblk tile).
                qkv lands in u[:, 0:512]; u[:, 512:1024] is transpose scratch."""
                xblk = xblk_pool.tile([P, CB, P], F32R, tag="xblk")
                nc.sync.dma_start(out=xblk, in_=xT[i])
                u = pair_ps.tile([P, 1024], F32, tag="pair")
                for cc in range(CB):
                    nc.tensor.matmul(u[:, 0:512], lhsT=xblk[:, cc, :],
                                     rhs=wqkvT_sb[:, cc, col0:col0 + C],
                                     start=(cc == 0), stop=(cc == CB - 1))
                return u, xblk

            def qk_finish(u, i, dstT, dst_nsl):
                """rope + transpose + evacuate u into dstT[:, :, dst_nsl]."""
                rot = rope_pool.tile([P, C], F32R, tag="qk")
                rope_block(u[:, 0:512], i, rot)
                ur = u[:].bitcast(F32R)
                for j in range(PAIRS):
                    nc.tensor.transpose(ur[:, 512 + j * P:512 + (j + 1) * P],
                                        rot[:, bass.ts(j, P)], identr)
                nc.vector.tensor_copy(
                    out=dstT[:, :, dst_nsl],
                    in_=ur[:, 512:1024].rearrange("p (j n) -> p j n", j=PAIRS))

            def v_mm(uv, half, i, xblk):
                for cc in range(CB):
                    nc.tensor.matmul(uv[:, half * 512:half * 512 + 512],
                                     lhsT=xblk[:, cc, :],
                                     rhs=wqkvT_sb[:, cc, 1024:1536],
                                     start=(cc == 0), stop=(cc == CB - 1))

            def a_block_pair(i0, dstT, nb0, with_v):
                """Blocks i0, i0+1: matmuls first, transposes after (keeps the
                in-order PE queue from stalling on the DVE rope)."""
                i1 = i0 + 1
                u0, x0 = qk_mm(i0, 512 if with_v else 0)
                u1, x1 = qk_mm(i1, 512 if with_v else 0)
                if with_v:
                    uv = num_pool.tile([P, 1024], F32, tag="num")
                    v_mm(uv, 0, i0, x0)
                    v_mm(uv, 1, i1, x1)
                qk_finish(u0, i0, dstT, bass.ts(nb0, P))
                qk_finish(u1, i1, dstT, bass.ts(nb0 + 1, P))
                if with_v:
                    nc.vector.tensor_copy(
                        out=vaug4[:, i0:i0 + 2, :, 0:D],
                        in_=uv[:].rearrange("p (b h d) -> p b h d", b=2, h=H))

            def body():
                # ---- stage A: k + v for all token blocks; q for the
                # first chunk-pair is interleaved so stage B can start as
                # soon as k/v are done ----
                qT0 = qT_pool.tile([P, PAIRS, 1024], F32R, tag="qT")
                for i2 in range(NB // 2):
                    i0 = 2 * i2
                    a_block_pair(i0, kT_sb, i0, True)
                    if i2 % 2 == 1 and i2 // 2 < 4 and "no_b" not in ablate:
                        j = i2 // 2
                        a_block_pair(2 * j, qT0, 2 * j, False)

                if "no_b" in ablate:
                    # keep-alive dumps so the compiler can not DCE stage A
                    nc.sync.dma_start(out=outT[0:P, :],
                                      in_=kT_sb[:, 0, :].bitcast(F32))
                    vv = singles.tile([P, NB, 128], F32)
                    nc.vector.tensor_copy(out=vv, in_=vaug_sb[:, :, 0:128])
                    nc.sync.dma_start(
                        out=outT[P:2 * P, :],
                        in_=vv[:].rearrange("p i e -> p (i e)"))
                    return

                # ---- two passes over query-chunk pairs (2 x 512 queries) ----
                for cp in range(2):
                    # q for the 8 token blocks of this chunk pair (pass 0's
                    # q was computed during the k/v loop)
                    if cp == 0:
                        qT_sb = qT0
                    else:
                        qT_sb = qT_pool.tile([P, PAIRS, 1024], F32R, tag="qT")
                        for j in range(4):
                            a_block_pair(cp * 8 + 2 * j, qT_sb, 2 * j, False)

                    attn0 = out_pool.tile([P, CB, 512], F32R, tag="attn")
                    attn1 = out_pool.tile([P, CB, 512], F32R, tag="attn")
                    for pj in range(PAIRS):
                        hA, hB = 2 * pj, 2 * pj + 1
                        num_A = num_pool.tile([P, 1024], F32, tag="num")
                        num_B = num_pool.tile([P, 1024], F32, tag="num")

                        def av(mb, eT0, eT1):
                            for h, numt, off in ((hA, num_A, 0), (hB, num_B, 512)):
                                for ci, ee in ((0, eT0), (1, eT1)):
                                    nc.tensor.matmul(
                                        numt[0:D + 1, ci * 512:ci * 512 + 512],
                                        lhsT=vaug_sb[:, mb, h * 65:(h + 1) * 65],
                                        rhs=ee[:, off:off + 512],
                                        start=(mb == 0), stop=(mb == NB - 1))

                        pend = []
                        for mb in range(NB):
                            p0 = pair_ps.tile([P, 1024], F32, tag="pair")
                            p1 = pair_ps.tile([P, 1024], F32, tag="pair")
                            for ci, pt in ((0, p0), (1, p1)):
                                nc.tensor.matmul(
                                    pt[:, 0:512],
                                    lhsT=kT_sb[0:64, pj, bass.ts(mb, P)],
                                    rhs=qT_sb[0:64, pj, ci * 512:ci * 512 + 512],
                                    start=True, stop=True,
                                    tile_position=(0, 0))
                                nc.tensor.matmul(
                                    pt[:, 512:1024],
                                    lhsT=kT_sb[64:128, pj, bass.ts(mb, P)],
                                    rhs=qT_sb[64:128, pj, ci * 512:ci * 512 + 512],
                                    start=True, stop=True,
                                    tile_position=(64, 0))
                            eT0 = eT_pool.tile([P, 1024], BF16, tag="eT")
                            eT1 = eT_pool.tile([P, 1024], BF16, tag="eT")
                            use_dve = (mb % 5 == 4) and s_act < NB
                            for pt, ee in ((p0, eT0), (p1, eT1)):
                                if use_dve:
                                    # Schraudolph exp on DVE:
                                    # bitcast(int16(z + A*mask+B))
                                    nc.vector.tensor_scalar(
                                        out=ee[:].bitcast(I16), in0=pt[:],
                                        scalar1=bprime[:, mb:mb + 1],
                                        scalar2=None, op0=AluOpType.add)
                                else:
                                    nc.scalar.activation(
                                        out=ee[:], in_=pt[:], func=EXP,
                                        bias=mask_sb[:, mb:mb + 1],
                                        scale=1.0 / A_SCHR)
                            pend.append((mb, eT0, eT1))
                            if len(pend) > 1:
                                av(*pend.pop(0))
                        for it in pend:
                            av(*it)
                        # normalize: row 64 of num = denominators
                        rcA = den_pool.tile([1, 1024], F32, tag="recip")
                        rcB = den_pool.tile([1, 1024], F32, tag="recip")
                        denA = den_pool.tile([64, 1024], F32, tag="den")
                        denB = den_pool.tile([64, 1024], F32, tag="den")
                        nc.vector.reciprocal(out=rcA, in_=num_A[64:65, :])
                        nc.vector.reciprocal(out=rcB, in_=num_B[64:65, :])
                        nc.gpsimd.partition_broadcast(denA[:], rcA[:])
                        nc.gpsimd.partition_broadcast(denB[:], rcB[:])
                        for ci, at in ((0, attn0), (1, attn1)):
                            nc.vector.tensor_tensor(
                                out=at[0:64, pj, :],
                                in0=num_A[0:64, ci * 512:ci * 512 + 512],
                                in1=denA[:, ci * 512:ci * 512 + 512],
                                op=AluOpType.mult)
                            nc.vector.tensor_tensor(
                                out=at[64:128, pj, :],
                                in0=num_B[0:64, ci * 512:ci * 512 + 512],
                                in1=denB[:, ci * 512:ci * 512 + 512],
                                op=AluOpType.mult)
                    # ---- stage C for both chunks of this pass ----
                    for ci, at in ((0, attn0), (1, attn1)):
                        qsl = bass.ts(2 * cp + ci, 512)
                        if "no_c" in ablate:
                            nc.sync.dma_start(out=outT[0:P, qsl],
                                              in_=at[:, 0, :].bitcast(F32))
                            continue
                        for ob2 in range(CB // 2):
                            uo = pair_ps.tile([P, 1024], F32, tag="pair")
                            for half in range(2):
                                ob = 2 * ob2 + half
                                for cc in range(CB):
                                    nc.tensor.matmul(
                                        uo[:, half * 512:half * 512 + 512],
                                        lhsT=wprojT_sb[:, cc, bass.ts(ob, P)],
                                        rhs=at[:, cc, :],
                                        start=(cc == 0), stop=(cc == CB - 1))
                            for half in range(2):
                                ob = 2 * ob2 + half
                                ot = out_pool.tile([P, 512], F32, tag="out")
                                nc.vector.tensor_scalar(
                                    out=ot[:],
                                    in0=uo[:, half * 512:half * 512 + 512],
                                    scalar1=bproj_sb[:, ob:ob + 1],
                                    scalar2=None, op0=AluOpType.add)
                                nc.sync.dma_start(out=outT[bass.ts(ob, P), qsl],
                                                  in_=ot[:])

            if repeats > 1:
                with tc.For_i(0, repeats) as _i:
                    body()
            else:
                body()

    nc.compile()
    return nc


def prep_in_maps(x, mask, W_qkv, W_proj, b_proj, num_cls_token):
    x = np.asarray(x, dtype=np.float32)
    mask = np.asarray(mask, dtype=np.float32)
    W_qkv = np.asarray(W_qkv, dtype=np.float32)
    W_proj = np.asarray(W_proj, dtype=np.float32)
    b_proj = np.asarray(b_proj, dtype=np.float32)
    ncls = int(np.asarray(num_cls_token))

    # De-interleave q/k head dims: within each head, rows [0,2,..,62, 1,3,..,63]
    # so rope operates on contiguous [un|ev] halves. Scores are invariant to
    # a shared q/k permutation. Scale k by A*0.125 for the Schraudolph path.
    perm64 = np.concatenate([np.arange(0, D, 2), np.arange(1, D, 2)])
    perm = np.arange(F)
    for sec in range(2):                       # q rows 0:512, k rows 512:1024
        for h in range(H):
            base = sec * C + h * D
            perm[base:base + D] = base + perm64
    Wp = W_qkv[perm]
    Wp = Wp.copy()
    Wp[C:2 * C] *= np.float32(A_SCHR * 0.125)   # fold A*scale into k
    wqkvT = np.ascontiguousarray(Wp.T)
    wprojT = np.ascontiguousarray(W_proj.T)

    # rope tables in de-interleaved order: j-th column is original dim 2j
    cos = np.ones((N, D // 2), dtype=np.float32)
    sin = np.zeros((N, D // 2), dtype=np.float32)
    if ncls < N:
        inv_freq = (1.0 / (10000.0 ** (np.arange(0, D, 2, dtype=np.float32)
                                       / np.float32(D)))).astype(np.float32)
        pos = np.arange(N - ncls, dtype=np.float32)
        freqs = pos[:, None] * inv_freq[None, :]
        cos[ncls:] = np.cos(freqs).astype(np.float32)
        sin[ncls:] = np.sin(freqs).astype(np.float32)
    sin_ext = np.concatenate([-sin, sin], axis=1)           # (N, 64)

    def part_major(a):
        return np.ascontiguousarray(
            a.reshape(CB, P, *a.shape[1:]).transpose(
                1, 0, *range(2, a.ndim + 1)))

    def tok_major(a):
        return np.ascontiguousarray(
            a.reshape(NB, P, *a.shape[1:]).transpose(
                1, 0, *range(2, a.ndim + 1)))

    wqkvT_h = part_major(wqkvT)
    wprojT_h = part_major(wprojT)
    bproj_h = part_major(b_proj)
    cos_h = tok_major(cos)
    sin_h = tok_major(sin_ext)

    in_maps = []
    for b in range(B):
        xb = x[b].reshape(NB, P, CB, P).transpose(0, 3, 2, 1)
        in_maps.append({
            "xT": np.ascontiguousarray(xb),
            "wqkvT": wqkvT_h,
            "wprojT": wprojT_h,
            "bproj": bproj_h,
            "maskd": tok_major(mask[b]),
            "cosd": cos_h,
            "sind": sin_h,
        })
    return in_maps


def gather_out(results):
    out = np.empty((B, N, C), dtype=np.float32)
    for b in range(B):
        out[b] = results[b]["outT"].T
    return out


_NC_CACHE = {}

TRACE = False          # test.py sets True to capture NTFF profile + exec time
LAST_RES = None        # BassKernelResults of the last kernel() call


def kernel(**inputs) -> np.ndarray:
    global LAST_RES
    from concourse.bass_utils import run_bass_kernel_spmd
    key = "single"
    if key not in _NC_CACHE:
        _NC_CACHE[key] = build_nc(repeats=1)
    nc = _NC_CACHE[key]
    in_maps = prep_in_maps(**inputs)
    res = run_bass_kernel_spmd(nc, in_maps, list(range(B)), trace=TRACE)
    LAST_RES = res
    return gather_out(res.results)
